# revision 44
# baseline (speedup 1.0000x reference)
"""Differential attention kernel for Trainium2, 8-core SPMD.

Math: the reference's two softmaxes collapse algebraically. With
k_prev = roll(k, +1, L), s_prev is a column-roll of s_cur, and softmax
commutes with column permutations, so
    a2 = roll(a1, +1, cols)  =>  o = a1 @ v_eff,
    v_eff = lam * (v - roll(v, -1, L)) = (x - roll(x, -1, L)) @ (lam*w_v).T
(the v-bias cancels in the difference). So the kernel is ONE standard
softmax attention with a modified value tensor. |s*scale| <= ~2.3 for
these inputs, so softmax runs without max-subtraction.

Sharding: core i handles batch i//4 and heads (i%4)*4..(i%4)*4+3
(data parallel on B, tensor parallel on heads; qkv col-split, out proj
row-split with partial sums reduced on host during the gather).
"""

import numpy as np
import ml_dtypes
import json as _json

import concourse.bacc as bacc
import concourse.tile as tile
from concourse import mybir
from concourse.bass_utils import run_bass_kernel_spmd

BF16 = mybir.dt.bfloat16
F32 = mybir.dt.float32
BFNP = ml_dtypes.bfloat16

B, D, H = 2, 1024, 16
DH = 64                # head dim
HPC = 4                # heads per core
HB = HPC * DH          # 256 head-block dims per core
N_CORES = 8
SCALE = 1.0 / 32.0     # d_model**-0.5

_nc_cache: dict = {}


def _dedup_ldweights(nc):
    """Drop Ldweights that reload the exact weights the PE already holds.

    Tile emits one Ldweights per matmul; back-to-back matmuls on the same
    stationary tile (e.g. the two 512-wide halves of one psum tile) reload
    identical weights. A reload is removable only if it carries no
    semaphore waits/updates and nothing but Matmults sit between it and
    the previous load in the PE stream.
    """
    removed = 0
    for fn in nc.m.functions:
        for blk in fn.blocks:
            prev_key = None
            keep = []
            for inst in blk.instructions:
                if isinstance(inst, mybir.InstLdweights):
                    key = _json.dumps(
                        _json.loads(mybir.instruction_to_pretty_json_string(
                            inst)).get("ins"))
                    si = inst.sync_info
                    clean = si is None or (not si.on_wait and not si.on_update)
                    if key == prev_key and clean:
                        removed += 1
                        continue
                    prev_key = key
                elif isinstance(inst, mybir.InstMatmult):
                    pass          # matmuls do not disturb loaded weights
                elif getattr(inst, "engine", None) == mybir.EngineType.PE:
                    prev_key = None   # other PE instruction: be conservative
                keep.append(inst)
            blk.instructions = keep
    return removed


def build_program(L: int = 2048):
    """Emit the single-core Bass/Tile program (same program on all cores)."""
    assert L % 128 == 0
    LT = L // 128                      # L tiles of 128
    QCH = min(L, 1024)                 # q chunk (ACT instr width / psum width)
    NQC = L // QCH                     # q chunks
    N512 = QCH // 512                  # 512-wide matmul slices per chunk
    DT = D // 128                      # 8 contraction tiles for the projections

    nc = bacc.Bacc("TRN2", target_bir_lowering=False, debug=False,
                   enable_asserts=False, num_devices=N_CORES)

    x_t = nc.dram_tensor("x_t", (DT, 128, L), BF16, kind="ExternalInput").ap()
    xd_t = nc.dram_tensor("xd_t", (DT, 128, L), BF16, kind="ExternalInput").ap()
    wqk_t = nc.dram_tensor("wqk_t", (D, 2 * HB), BF16, kind="ExternalInput").ap()
    wvl_t = nc.dram_tensor("wvl_t", (D, HB), BF16, kind="ExternalInput").ap()
    bqk = nc.dram_tensor("bqk", (4, 128), F32, kind="ExternalInput").ap()
    wout_t = nc.dram_tensor("wout_t", (HB, D), BF16, kind="ExternalInput").ap()
    out_p = nc.dram_tensor("out_p", (L, D), BF16, kind="ExternalOutput").ap()

    with tile.TileContext(nc) as tc:
        with (
            tc.tile_pool(name="const", bufs=1) as const,
            tc.tile_pool(name="psum_big", bufs=2, space="PSUM") as psum_big,
            tc.tile_pool(name="psum_o", bufs=1, space="PSUM") as psum_o,
            tc.tile_pool(name="psum_proj", bufs=1, space="PSUM") as psum_proj,
            tc.tile_pool(name="pbuf", bufs=4) as pbuf,
            tc.tile_pool(name="ostage", bufs=2) as ostage,
            tc.tile_pool(name="outbuf", bufs=3) as outbuf,
            tc.tile_pool(name="misc", bufs=2) as misc,
            tc.tile_pool(name="dramp", bufs=2, space="DRAM") as dramp,
        ):
            # ---- persistent SBUF tensors -------------------------------
            # x/xd as separate per-d-tile tensors so the qkv matmuls can
            # start as soon as the first chunks land (per-tile DMA deps);
            # input DMAs split across two HWDGE queues (sync + scalar)
            wqk_sb = const.tile([128, DT, 2 * HB], BF16)
            nc.sync.dma_start(out=wqk_sb,
                              in_=wqk_t.rearrange("(t p) m -> p t m", p=128))
            bqk_sb = const.tile([128, 4], F32)
            nc.scalar.dma_start(out=bqk_sb, in_=bqk.rearrange("t p -> p t"))
            x_sb = []
            for dd in range(DT):
                xt_d = const.tile([128, L], BF16, name=f"x_sb{dd}")
                eng = nc.sync if dd % 2 == 0 else nc.scalar
                eng.dma_start(out=xt_d, in_=x_t[dd])
                x_sb.append(xt_d)
            xd_sb = []
            for dd in range(DT):
                xd_d = const.tile([128, L], BF16, name=f"xd_sb{dd}")
                eng = nc.sync if dd % 2 == 0 else nc.scalar
                eng.dma_start(out=xd_d, in_=xd_t[dd])
                xd_sb.append(xd_d)
            wvl_sb = const.tile([128, DT, HB], BF16)
            nc.scalar.dma_start(out=wvl_sb,
                                in_=wvl_t.rearrange("(t p) m -> p t m", p=128))
            wout_sb = const.tile([128, 2, D], BF16)
            nc.scalar.dma_start(out=wout_sb,
                                in_=wout_t.rearrange("(t p) n -> p t n", p=128))

            # q.T/k.T per m-tile: 0,1 = q dims 0..255; 2,3 = k dims 0..255
            qk_sb = [const.tile([128, L], BF16, name=f"qk_sb{m}")
                     for m in range(4)]
            # v_ext per lk-tile: [head, 64 v dims + ones column]
            vext_sb = []
            for lt in range(LT):
                vx = const.tile([128, HPC, DH + 1], BF16, name=f"vext{lt}")
                nc.vector.memset(vx[:, :, DH:DH + 1], 1.0)
                vext_sb.append(vx)
            # normalized o.T (o dims on partitions, head-major across ptiles)
            onorm_sb = const.tile([128, 2, L], BF16)

            # ---- projections -------------------------------------------
            MMN = min(L, 1024)

            def qkv_mtile(m):
                """qk.T m-tile: psum = W_qk[:,m] @ x.T, evict + bias."""
                for half in range(max(1, L // MMN)):
                    ps = psum_proj.tile([128, MMN], F32, tag="proj",
                                        name=f"qk_ps_{m}_{half}")
                    for d in range(DT):
                        lhsT = wqk_sb[:, d, m * 128:(m + 1) * 128]
                        for n in range(MMN // 512):
                            nc.tensor.matmul(
                                ps[:, n * 512:(n + 1) * 512], lhsT,
                                x_sb[d][:, half * MMN + n * 512:
                                        half * MMN + (n + 1) * 512],
                                start=(d == 0), stop=(d == DT - 1))
                    nc.vector.tensor_scalar_add(
                        qk_sb[m][:, half * MMN:(half + 1) * MMN],
                        ps, bqk_sb[:, m:m + 1])

            def vl_tile(lt):
                """v_eff l-tile into v_ext columns."""
                psv = psum_proj.tile([128, HB], F32, tag="proj",
                                     name=f"vl_{lt}")
                for d in range(DT):
                    nc.tensor.matmul(
                        psv, xd_sb[d][:, lt * 128:(lt + 1) * 128],
                        wvl_sb[:, d, :], start=(d == 0), stop=(d == DT - 1))
                nc.vector.tensor_copy(
                    vext_sb[lt][:, :, 0:DH],
                    psv.rearrange("p (h c) -> p h c", c=DH))

            qkv_mtile(0)
            qkv_mtile(2)
            for lt in range(LT):
                vl_tile(lt)

            # ---- attention per (head, q chunk) -------------------------
            # m-tiles 1,3 (heads 2,3) are emitted after h1 so their matmuls
            # fill PE gaps during h0/h1's ACT-bound attention
            for h in range(HPC):
                if h == 2:
                    qkv_mtile(1)
                    qkv_mtile(3)
                po = 64 * (h % 2)          # partition offset of this head
                mt = h // 2                # q/k ptile index
                for qc in range(NQC):
                    o_ps = psum_o.tile([DH + 1, QCH], F32, tag="o")
                    for kt in range(LT):
                        s_ps = psum_big.tile([128, QCH], F32, tag="big")
                        k_st = qk_sb[2 + mt][po:po + DH,
                                             kt * 128:(kt + 1) * 128]
                        for n in range(N512):
                            nc.tensor.matmul(
                                s_ps[:, n * 512:(n + 1) * 512], k_st,
                                qk_sb[mt][po:po + DH,
                                          qc * QCH + n * 512:
                                          qc * QCH + (n + 1) * 512],
                                start=True, stop=True)
                        p_sb = pbuf.tile([128, QCH], BF16, tag="p")
                        nc.scalar.activation(
                            p_sb, s_ps, mybir.ActivationFunctionType.Exp,
                            scale=SCALE)
                        vext = vext_sb[kt][:, h, :]
                        for n in range(N512):
                            nc.tensor.matmul(
                                o_ps[:, n * 512:(n + 1) * 512], vext,
                                p_sb[:, n * 512:(n + 1) * 512],
                                start=(kt == 0), stop=(kt == LT - 1))
                    # free the psum accumulator fast: one copy to SBUF, then
                    # normalize entirely from the staging copy
                    ost = ostage.tile([DH + 1, QCH], F32, tag="ost")
                    nc.vector.tensor_copy(ost, o_ps)
                    # reciprocal is free-size-bound on DVE, so transpose the
                    # denom row into [128, QCH/128] via a DRAM bounce first
                    d_dram = dramp.tile([QCH], F32, tag="dd")
                    nc.sync.dma_start(out=d_dram, in_=ost[DH:DH + 1, :])
                    dtp = misc.tile([128, QCH // 128], F32, tag="dtp")
                    nc.sync.dma_start(
                        out=dtp, in_=d_dram.rearrange("(p f) -> p f", p=128))
                    rtp = misc.tile([128, QCH // 128], F32, tag="rtp")
                    nc.vector.reciprocal(rtp, dtp)
                    r_dram = dramp.tile([QCH], F32, tag="rd")
                    nc.sync.dma_start(
                        out=r_dram.rearrange("(p f) -> p f", p=128), in_=rtp)
                    rbc = misc.tile([DH, QCH], F32, tag="rbc")
                    nc.gpsimd.dma_start(
                        out=rbc, in_=r_dram[:].partition_broadcast(DH))
                    nc.vector.tensor_mul(
                        onorm_sb[po:po + DH, mt, qc * QCH:(qc + 1) * QCH],
                        ost[0:DH, :], rbc)

            # ---- out projection: out_p = o_norm.T.T @ w_out_slice.T ----
            for qt in range(LT):
                pso = psum_big.tile([128, D], F32, tag="big")
                for kk in range(2):
                    lhsT = onorm_sb[:, kk, qt * 128:(qt + 1) * 128]
                    for n in range(D // 512):
                        nc.tensor.matmul(
                            pso[:, n * 512:(n + 1) * 512], lhsT,
                            wout_sb[:, kk, n * 512:(n + 1) * 512],
                            start=(kk == 0), stop=(kk == 1))
                ot = outbuf.tile([128, D], BF16, tag="ot")
                nc.vector.tensor_copy(ot, pso)
                nc.sync.dma_start(
                    out=out_p.rearrange("(t p) n -> t p n", p=128)[qt], in_=ot)

    import os as _os
    if _os.environ.get("NO_DEDUP", "0") != "1":
        _dedup_ldweights(nc)
    nc.compile()   # bacc passes: reg alloc, act table loads, nop fusion
    return nc


def _get_nc(L: int = 2048):
    if L not in _nc_cache:
        _nc_cache[L] = build_program(L)
    return _nc_cache[L]


def prep_in_maps(x, w_qkv, b_qkv, w_out, lam):
    """Host-side sharding: slice/transpose/cast per-core inputs."""
    x = np.asarray(x, dtype=np.float32)
    w_qkv = np.asarray(w_qkv, dtype=np.float32)
    b_qkv = np.asarray(b_qkv, dtype=np.float32)
    w_out = np.asarray(w_out, dtype=np.float32)
    lam = float(lam)

    def pack_x(a_t):      # [D, L] -> [DT, 128, L] bf16
        d, n = a_t.shape
        return np.ascontiguousarray(a_t.reshape(d // 128, 128, n)).astype(BFNP)

    x_t_b = [pack_x(x[b].T) for b in range(B)]
    xd = x - np.roll(x, -1, axis=1)
    xd_t_b = [pack_x(xd[b].T) for b in range(B)]

    in_maps = []
    for core in range(N_CORES):
        b = core // 4
        r0 = (core % 4) * HB
        wq = w_qkv[r0:r0 + HB]
        wk = w_qkv[D + r0:D + r0 + HB]
        wv = lam * w_qkv[2 * D + r0:2 * D + r0 + HB]
        in_maps.append({
            "x_t": x_t_b[b],
            "xd_t": xd_t_b[b],
            "wqk_t": np.ascontiguousarray(
                np.concatenate([wq, wk], axis=0).T).astype(BFNP),
            "wvl_t": np.ascontiguousarray(wv.T).astype(BFNP),
            "bqk": np.concatenate(
                [b_qkv[r0:r0 + HB], b_qkv[D + r0:D + r0 + HB]]
            ).astype(np.float32).reshape(4, 128),
            "wout_t": np.ascontiguousarray(
                w_out[:, r0:r0 + HB].T).astype(BFNP),
        })
    return in_maps


def run_device(in_maps, trace=False, trace_cores=None):
    nc = _get_nc()
    return run_bass_kernel_spmd(
        nc, in_maps, core_ids=list(range(N_CORES)),
        trace=trace, trace_cores=trace_cores)


def gather_output(results, b_out):
    out = np.zeros((B, 2048, D), dtype=np.float32)
    for core in range(N_CORES):
        out[core // 4] += np.asarray(results[core]["out_p"], dtype=np.float32)
    out += np.asarray(b_out, dtype=np.float32)[None, None, :]
    return out


def kernel(x, w_qkv, b_qkv, w_out, b_out, lam, heads=H, **_ignored):
    assert int(heads) == H
    in_maps = prep_in_maps(x, w_qkv, b_qkv, w_out, lam)
    br = run_device(in_maps, trace=False)
    return gather_output(br.results, b_out)


# revision 45
# speedup vs baseline: 1.0127x; 1.0127x over previous
"""Differential attention kernel for Trainium2, 8-core SPMD.

Math: the reference's two softmaxes collapse algebraically. With
k_prev = roll(k, +1, L), s_prev is a column-roll of s_cur, and softmax
commutes with column permutations, so
    a2 = roll(a1, +1, cols)  =>  o = a1 @ v_eff,
    v_eff = lam * (v - roll(v, -1, L)) = (x - roll(x, -1, L)) @ (lam*w_v).T
(the v-bias cancels in the difference). So the kernel is ONE standard
softmax attention with a modified value tensor. |s*scale| <= ~2.3 for
these inputs, so softmax runs without max-subtraction.

Sharding: core i handles batch i//4 and heads (i%4)*4..(i%4)*4+3
(data parallel on B, tensor parallel on heads; qkv col-split, out proj
row-split with partial sums reduced on host during the gather).
"""

import numpy as np
import ml_dtypes
import json as _json

import concourse.bacc as bacc
import concourse.tile as tile
from concourse import mybir
from concourse.bass_utils import run_bass_kernel_spmd

BF16 = mybir.dt.bfloat16
F32 = mybir.dt.float32
BFNP = ml_dtypes.bfloat16

B, D, H = 2, 1024, 16
DH = 64                # head dim
HPC = 4                # heads per core
HB = HPC * DH          # 256 head-block dims per core
N_CORES = 8
SCALE = 1.0 / 32.0     # d_model**-0.5

_nc_cache: dict = {}


def _dedup_ldweights(nc):
    """Drop Ldweights that reload the exact weights the PE already holds.

    Tile emits one Ldweights per matmul; back-to-back matmuls on the same
    stationary tile (e.g. the two 512-wide halves of one psum tile) reload
    identical weights. A reload is removable only if it carries no
    semaphore waits/updates and nothing but Matmults sit between it and
    the previous load in the PE stream.
    """
    removed = 0
    for fn in nc.m.functions:
        for blk in fn.blocks:
            prev_key = None
            keep = []
            for inst in blk.instructions:
                if isinstance(inst, mybir.InstLdweights):
                    key = _json.dumps(
                        _json.loads(mybir.instruction_to_pretty_json_string(
                            inst)).get("ins"))
                    si = inst.sync_info
                    clean = si is None or (not si.on_wait and not si.on_update)
                    if key == prev_key and clean:
                        removed += 1
                        continue
                    prev_key = key
                elif isinstance(inst, mybir.InstMatmult):
                    pass          # matmuls do not disturb loaded weights
                elif getattr(inst, "engine", None) == mybir.EngineType.PE:
                    prev_key = None   # other PE instruction: be conservative
                keep.append(inst)
            blk.instructions = keep
    return removed


def build_program(L: int = 2048):
    """Emit the single-core Bass/Tile program (same program on all cores)."""
    assert L % 128 == 0
    LT = L // 128                      # L tiles of 128
    QCH = min(L, 1024)                 # q chunk (ACT instr width / psum width)
    NQC = L // QCH                     # q chunks
    N512 = QCH // 512                  # 512-wide matmul slices per chunk
    DT = D // 128                      # 8 contraction tiles for the projections

    nc = bacc.Bacc("TRN2", target_bir_lowering=False, debug=False,
                   enable_asserts=False, num_devices=N_CORES)

    x_t = nc.dram_tensor("x_t", (DT, 128, L), BF16, kind="ExternalInput").ap()
    xd_t = nc.dram_tensor("xd_t", (DT, 128, L), BF16, kind="ExternalInput").ap()
    wqk_t = nc.dram_tensor("wqk_t", (D, 2 * HB), BF16, kind="ExternalInput").ap()
    wvl_t = nc.dram_tensor("wvl_t", (D, HB), BF16, kind="ExternalInput").ap()
    bqk = nc.dram_tensor("bqk", (4, 128), F32, kind="ExternalInput").ap()
    wout_t = nc.dram_tensor("wout_t", (HB, D), BF16, kind="ExternalInput").ap()
    out_p = nc.dram_tensor("out_p", (L, D), BF16, kind="ExternalOutput").ap()

    with tile.TileContext(nc) as tc:
        with (
            tc.tile_pool(name="const", bufs=1) as const,
            tc.tile_pool(name="psum_big", bufs=2, space="PSUM") as psum_big,
            tc.tile_pool(name="psum_o", bufs=1, space="PSUM") as psum_o,
            tc.tile_pool(name="psum_proj", bufs=1, space="PSUM") as psum_proj,
            tc.tile_pool(name="pbuf", bufs=4) as pbuf,
            tc.tile_pool(name="ostage", bufs=2) as ostage,
            tc.tile_pool(name="outbuf", bufs=3) as outbuf,
            tc.tile_pool(name="misc", bufs=2) as misc,
            tc.tile_pool(name="dramp", bufs=2, space="DRAM") as dramp,
        ):
            # ---- persistent SBUF tensors -------------------------------
            # x/xd as separate per-d-tile tensors so the qkv matmuls can
            # start as soon as the first chunks land (per-tile DMA deps);
            # input DMAs split across two HWDGE queues (sync + scalar)
            wqk_dv = wqk_t.rearrange("(t p) m -> t p m", p=128)
            wqk_sb = []
            for dd in range(DT):
                wq_d = const.tile([128, 2 * HB], BF16, name=f"wqk_sb{dd}")
                nc.sync.dma_start(out=wq_d, in_=wqk_dv[dd])
                wqk_sb.append(wq_d)
            bqk_sb = const.tile([128, 4], F32)
            nc.scalar.dma_start(out=bqk_sb, in_=bqk.rearrange("t p -> p t"))
            x_sb = []
            for dd in range(DT):
                xt_d = const.tile([128, L], BF16, name=f"x_sb{dd}")
                eng = nc.sync if dd % 2 == 0 else nc.scalar
                eng.dma_start(out=xt_d, in_=x_t[dd])
                x_sb.append(xt_d)
            xd_sb = []
            for dd in range(DT):
                xd_d = const.tile([128, L], BF16, name=f"xd_sb{dd}")
                eng = nc.sync if dd % 2 == 0 else nc.scalar
                eng.dma_start(out=xd_d, in_=xd_t[dd])
                xd_sb.append(xd_d)
            wvl_sb = const.tile([128, DT, HB], BF16)
            nc.scalar.dma_start(out=wvl_sb,
                                in_=wvl_t.rearrange("(t p) m -> p t m", p=128))
            wout_sb = const.tile([128, 2, D], BF16)
            nc.scalar.dma_start(out=wout_sb,
                                in_=wout_t.rearrange("(t p) n -> p t n", p=128))

            # q.T/k.T per m-tile: 0,1 = q dims 0..255; 2,3 = k dims 0..255
            qk_sb = [const.tile([128, L], BF16, name=f"qk_sb{m}")
                     for m in range(4)]
            # v_ext per lk-tile: [head, 64 v dims + ones column]
            vext_sb = []
            for lt in range(LT):
                vx = const.tile([128, HPC, DH + 1], BF16, name=f"vext{lt}")
                nc.vector.memset(vx[:, :, DH:DH + 1], 1.0)
                vext_sb.append(vx)
            # normalized o.T (o dims on partitions, head-major across ptiles)
            onorm_sb = const.tile([128, 2, L], BF16)

            # ---- projections -------------------------------------------
            MMN = min(L, 1024)

            def qkv_mtile(m, tag="big"):
                """qk.T m-tile: psum = W_qk[:,m] @ x.T, evict + bias."""
                for half in range(max(1, L // MMN)):
                    ps = psum_proj.tile([128, MMN], F32, tag=tag,
                                        name=f"qk_ps_{m}_{half}") \
                        if tag == "proj" else \
                        psum_big.tile([128, MMN], F32, tag=tag,
                                      name=f"qk_ps_{m}_{half}")
                    for d in range(DT):
                        lhsT = wqk_sb[d][:, m * 128:(m + 1) * 128]
                        for n in range(MMN // 512):
                            nc.tensor.matmul(
                                ps[:, n * 512:(n + 1) * 512], lhsT,
                                x_sb[d][:, half * MMN + n * 512:
                                        half * MMN + (n + 1) * 512],
                                start=(d == 0), stop=(d == DT - 1))
                    nc.vector.tensor_scalar_add(
                        qk_sb[m][:, half * MMN:(half + 1) * MMN],
                        ps, bqk_sb[:, m:m + 1])

            def vl_tile(lt):
                """v_eff l-tile into v_ext columns."""
                psv = psum_big.tile([128, HB], F32, tag="big",
                                     name=f"vl_{lt}")
                for d in range(DT):
                    nc.tensor.matmul(
                        psv, xd_sb[d][:, lt * 128:(lt + 1) * 128],
                        wvl_sb[:, d, :], start=(d == 0), stop=(d == DT - 1))
                nc.vector.tensor_copy(
                    vext_sb[lt][:, :, 0:DH],
                    psv.rearrange("p (h c) -> p h c", c=DH))

            qkv_mtile(0)
            qkv_mtile(2)
            for lt in range(LT):
                vl_tile(lt)

            # ---- attention per (head, q chunk) -------------------------
            # m-tiles 1,3 (heads 2,3) are emitted after h1 so their matmuls
            # fill PE gaps during h0/h1's ACT-bound attention
            for h in range(HPC):
                if h == 2:
                    qkv_mtile(1, tag="proj")
                    qkv_mtile(3, tag="proj")
                po = 64 * (h % 2)          # partition offset of this head
                mt = h // 2                # q/k ptile index
                for qc in range(NQC):
                    o_ps = psum_o.tile([DH + 1, QCH], F32, tag="o")
                    for kt in range(LT):
                        s_ps = psum_big.tile([128, QCH], F32, tag="big")
                        k_st = qk_sb[2 + mt][po:po + DH,
                                             kt * 128:(kt + 1) * 128]
                        for n in range(N512):
                            nc.tensor.matmul(
                                s_ps[:, n * 512:(n + 1) * 512], k_st,
                                qk_sb[mt][po:po + DH,
                                          qc * QCH + n * 512:
                                          qc * QCH + (n + 1) * 512],
                                start=True, stop=True)
                        p_sb = pbuf.tile([128, QCH], BF16, tag="p")
                        nc.scalar.activation(
                            p_sb, s_ps, mybir.ActivationFunctionType.Exp,
                            scale=SCALE)
                        vext = vext_sb[kt][:, h, :]
                        for n in range(N512):
                            nc.tensor.matmul(
                                o_ps[:, n * 512:(n + 1) * 512], vext,
                                p_sb[:, n * 512:(n + 1) * 512],
                                start=(kt == 0), stop=(kt == LT - 1))
                    # free the psum accumulator fast: one copy to SBUF, then
                    # normalize entirely from the staging copy
                    ost = ostage.tile([DH + 1, QCH], F32, tag="ost")
                    nc.vector.tensor_copy(ost, o_ps)
                    # reciprocal is free-size-bound on DVE, so transpose the
                    # denom row into [128, QCH/128] via a DRAM bounce first
                    d_dram = dramp.tile([QCH], F32, tag="dd")
                    nc.sync.dma_start(out=d_dram, in_=ost[DH:DH + 1, :])
                    dtp = misc.tile([128, QCH // 128], F32, tag="dtp")
                    nc.sync.dma_start(
                        out=dtp, in_=d_dram.rearrange("(p f) -> p f", p=128))
                    rtp = misc.tile([128, QCH // 128], F32, tag="rtp")
                    nc.vector.reciprocal(rtp, dtp)
                    r_dram = dramp.tile([QCH], F32, tag="rd")
                    nc.sync.dma_start(
                        out=r_dram.rearrange("(p f) -> p f", p=128), in_=rtp)
                    rbc = misc.tile([DH, QCH], F32, tag="rbc")
                    nc.gpsimd.dma_start(
                        out=rbc, in_=r_dram[:].partition_broadcast(DH))
                    nc.vector.tensor_mul(
                        onorm_sb[po:po + DH, mt, qc * QCH:(qc + 1) * QCH],
                        ost[0:DH, :], rbc)

            # ---- out projection: out_p = o_norm.T.T @ w_out_slice.T ----
            for qt in range(LT):
                pso = psum_big.tile([128, D], F32, tag="big")
                for kk in range(2):
                    lhsT = onorm_sb[:, kk, qt * 128:(qt + 1) * 128]
                    for n in range(D // 512):
                        nc.tensor.matmul(
                            pso[:, n * 512:(n + 1) * 512], lhsT,
                            wout_sb[:, kk, n * 512:(n + 1) * 512],
                            start=(kk == 0), stop=(kk == 1))
                ot = outbuf.tile([128, D], BF16, tag="ot")
                nc.vector.tensor_copy(ot, pso)
                nc.sync.dma_start(
                    out=out_p.rearrange("(t p) n -> t p n", p=128)[qt], in_=ot)

    # note: _dedup_ldweights is UNSAFE on hardware (exec-unit fault seen
    # with the split-tile layout) and measured ~0 gain -- LDWs pipeline.
    nc.compile()   # bacc passes: reg alloc, act table loads, nop fusion
    return nc


def _get_nc(L: int = 2048):
    if L not in _nc_cache:
        _nc_cache[L] = build_program(L)
    return _nc_cache[L]


def prep_in_maps(x, w_qkv, b_qkv, w_out, lam):
    """Host-side sharding: slice/transpose/cast per-core inputs."""
    x = np.asarray(x, dtype=np.float32)
    w_qkv = np.asarray(w_qkv, dtype=np.float32)
    b_qkv = np.asarray(b_qkv, dtype=np.float32)
    w_out = np.asarray(w_out, dtype=np.float32)
    lam = float(lam)

    def pack_x(a_t):      # [D, L] -> [DT, 128, L] bf16
        d, n = a_t.shape
        return np.ascontiguousarray(a_t.reshape(d // 128, 128, n)).astype(BFNP)

    x_t_b = [pack_x(x[b].T) for b in range(B)]
    xd = x - np.roll(x, -1, axis=1)
    xd_t_b = [pack_x(xd[b].T) for b in range(B)]

    in_maps = []
    for core in range(N_CORES):
        b = core // 4
        r0 = (core % 4) * HB
        wq = w_qkv[r0:r0 + HB]
        wk = w_qkv[D + r0:D + r0 + HB]
        wv = lam * w_qkv[2 * D + r0:2 * D + r0 + HB]
        in_maps.append({
            "x_t": x_t_b[b],
            "xd_t": xd_t_b[b],
            "wqk_t": np.ascontiguousarray(
                np.concatenate([wq, wk], axis=0).T).astype(BFNP),
            "wvl_t": np.ascontiguousarray(wv.T).astype(BFNP),
            "bqk": np.concatenate(
                [b_qkv[r0:r0 + HB], b_qkv[D + r0:D + r0 + HB]]
            ).astype(np.float32).reshape(4, 128),
            "wout_t": np.ascontiguousarray(
                w_out[:, r0:r0 + HB].T).astype(BFNP),
        })
    return in_maps


def run_device(in_maps, trace=False, trace_cores=None):
    nc = _get_nc()
    return run_bass_kernel_spmd(
        nc, in_maps, core_ids=list(range(N_CORES)),
        trace=trace, trace_cores=trace_cores)


def gather_output(results, b_out):
    out = np.zeros((B, 2048, D), dtype=np.float32)
    for core in range(N_CORES):
        out[core // 4] += np.asarray(results[core]["out_p"], dtype=np.float32)
    out += np.asarray(b_out, dtype=np.float32)[None, None, :]
    return out


def kernel(x, w_qkv, b_qkv, w_out, b_out, lam, heads=H, **_ignored):
    assert int(heads) == H
    in_maps = prep_in_maps(x, w_qkv, b_qkv, w_out, lam)
    br = run_device(in_maps, trace=False)
    return gather_output(br.results, b_out)
